# revision 7
# baseline (speedup 1.0000x reference)
"""Trainium2 Bass kernel for nn_Decoder (moe_routing): 4-layer decoder with
self-attn + cross-attn + top-2-of-8 MoE per layer.

Distribution (8 NeuronCores): token-sharded. Core c owns 256 tokens
(cores 0-3 = batch 0, cores 4-7 = batch 1, chunk q = c % 4). Cross-attention
and the MoE are fully token-local. Self-attention needs full-batch K/V, so
each layer starts with one grouped AllGather of y over the 4 cores of each
batch; K/V projections are recomputed per core (causality handled by an
additive mask, so all cores run an identical program).

The MoE is computed densely (all 8 experts on the core's 256 tokens) in fp32
and combined with the renormalized top-2 gates, exactly mirroring the
reference math. Expert weights stream from HBM (64 MB/layer/core).

All matmuls are plain fp32: the router's top-2 margins go down to ~1e-5, so
tf32-like fast modes (fp32r/bf16) flip expert selection and are not usable.

Host side: embedding + positional lookup, input layout prep (transposes,
concatenated K/V weights, mask construction), final output assembly and the
load-balancing aux loss (computed from the device-produced router probs).
"""
import os
import numpy as np

import concourse.bass as bass
import concourse.mybir as mybir
import concourse.tile as tile
from concourse import bacc
from concourse.bass_utils import run_bass_kernel_spmd
from concourse.masks import make_identity

F32 = mybir.dt.float32
AF = mybir.ActivationFunctionType
ALU = mybir.AluOpType

B, T, S = 2, 1024, 1024
D, H, L = 512, 8, 4
E, K, FF = 8, 2, 2048
DH = D // H
EPS = 1e-5
NCHUNK = 256            # tokens per core
TB = 1024               # tokens per batch
N_CORES = 8
GROUPS = [[0, 1, 2, 3], [4, 5, 6, 7]]

DEBUG = bool(int(os.environ.get("KERNEL_DEBUG", "0")))
N_LAYERS = int(os.environ.get("KERNEL_LAYERS", str(L)))

_cache = {}


def _ln(nc, pools, out_ap, in_ap, g_bcast, b_bcast, stats_pool):
    """LayerNorm over free dim (512) of [128, 512] in_ap -> out_ap."""
    st = stats_pool.tile([128, 6], F32, tag="st")
    nc.vector.bn_stats(out=st, in_=in_ap)
    mv = stats_pool.tile([128, 2], F32, tag="mv")
    nc.vector.bn_aggr(out=mv, in_=st)
    sq = stats_pool.tile([128, 1], F32, tag="sq")
    nc.scalar.activation(out=sq, in_=mv[:, 1:2], func=AF.Sqrt,
                         bias=pools["eps"][:, 0:1], scale=1.0)
    rstd = stats_pool.tile([128, 1], F32, tag="rstd")
    nc.vector.reciprocal(out=rstd, in_=sq)
    nc.vector.tensor_scalar(out=in_ap, in0=in_ap, scalar1=mv[:, 0:1], scalar2=rstd,
                            op0=ALU.subtract, op1=ALU.mult)
    nc.vector.tensor_mul(out=in_ap, in0=in_ap, in1=g_bcast)
    nc.vector.tensor_add(out=out_ap, in0=in_ap, in1=b_bcast)


def build(n_layers=N_LAYERS, debug=DEBUG):
    nc = bacc.Bacc(None, target_bir_lowering=False, num_devices=N_CORES)

    # ---- I/O ----
    inp = {}
    def din(name, shape, dtype=F32):
        inp[name] = nc.dram_tensor(name, list(shape), dtype, kind="ExternalInput")
        return inp[name]

    y0_chunk = din("y0_chunk", [NCHUNK, D])
    y0T_b = din("y0T_b", [D, TB])
    mask_in = din("mask", [NCHUNK, TB])
    encT_in = din("encT_b", [D, S])
    wkv_sa = din("wkv_sa", [L, D, 2 * D]); bkv_sa = din("bkv_sa", [L, 2 * D])
    wq_sa = din("wq_sa", [L, D, D]); bq_sa = din("bq_sa", [L, D])
    wo_sa = din("wo_sa", [L, D, D]); bo_sa = din("bo_sa", [L, D])
    wkv_ca = din("wkv_ca", [L, D, 2 * D]); bkv_ca = din("bkv_ca", [L, 2 * D])
    wq_ca = din("wq_ca", [L, D, D]); bq_ca = din("bq_ca", [L, D])
    wo_ca = din("wo_ca", [L, D, D]); bo_ca = din("bo_ca", [L, D])
    ln_g = din("ln_g", [L, 3, D]); ln_b = din("ln_b", [L, 3, D])
    rw = din("rw", [L, D, E]); rb = din("rb", [L, E])
    ew1 = din("ew1", [L, E, D, FF]); eb1 = din("eb1", [L, E, FF])
    ew2 = din("ew2", [L, E, FF, D]); eb2 = din("eb2", [L, E, D])

    out_y = nc.dram_tensor("out_y", [NCHUNK, D], F32, kind="ExternalOutput")
    probs_out = nc.dram_tensor("probs_out", [L, NCHUNK, E], F32, kind="ExternalOutput")
    if debug:
        dbg = nc.dram_tensor("dbg", [L, 3, NCHUNK, D], F32, kind="ExternalOutput")

    with tile.TileContext(nc) as tc:
        import contextlib
        ctx = contextlib.ExitStack()
        with ctx:
            const = ctx.enter_context(tc.tile_pool(name="const", bufs=1))
            ypool = ctx.enter_context(tc.tile_pool(name="ypool", bufs=2))
            ycTp = ctx.enter_context(tc.tile_pool(name="ycTp", bufs=2))
            ybTp = ctx.enter_context(tc.tile_pool(name="ybTp", bufs=1))
            kTp = ctx.enter_context(tc.tile_pool(name="kTp", bufs=1))
            vtokp = ctx.enter_context(tc.tile_pool(name="vtokp", bufs=1))
            wproj = ctx.enter_context(tc.tile_pool(name="wproj", bufs=2))
            smax = ctx.enter_context(tc.tile_pool(name="smax", bufs=3))
            qavp = ctx.enter_context(tc.tile_pool(name="qavp", bufs=2))
            big2 = ctx.enter_context(tc.tile_pool(name="big2", bufs=1))
            stream = ctx.enter_context(tc.tile_pool(name="stream", bufs=2))
            maccp = ctx.enter_context(tc.tile_pool(name="maccp", bufs=1))
            bcast = ctx.enter_context(tc.tile_pool(name="bcast", bufs=3))
            small = ctx.enter_context(tc.tile_pool(name="small", bufs=4))
            dramp = ctx.enter_context(tc.tile_pool(name="dramp", bufs=2, space="DRAM"))
            psA = ctx.enter_context(tc.tile_pool(name="psA", bufs=2, space="PSUM"))
            psB = ctx.enter_context(tc.tile_pool(name="psB", bufs=2, space="PSUM"))
            psC = ctx.enter_context(tc.tile_pool(name="psC", bufs=2, space="PSUM"))

            pools = {}

            ident = const.tile([128, 128], F32)
            make_identity(nc, ident)
            eps_t = const.tile([128, 1], F32)
            nc.vector.memset(eps_t, EPS)
            pools["eps"] = eps_t

            mask_t = const.tile([128, 2, TB], F32)
            nc.sync.dma_start(out=mask_t, in_=mask_in.rearrange("(c p) t -> p c t", p=128))
            encT = const.tile([128, 4, S], F32)
            nc.sync.dma_start(out=encT, in_=encT_in.rearrange("(kc p) t -> p kc t", p=128))

            y_chunk = ypool.tile([128, 2, D], F32, tag="y")
            nc.sync.dma_start(out=y_chunk, in_=y0_chunk.rearrange("(c p) d -> p c d", p=128))

            def transpose_chunk(src_y):
                """[128, 2, 512] token-major -> ycT [128, 4, 256] feature-major."""
                ycT = ycTp.tile([128, 4, NCHUNK], F32, tag="ycT")
                for kc in range(4):
                    ps = psC.tile([128, 256], F32, tag="c")
                    for c in range(2):
                        nc.tensor.transpose(ps[:, c * 128:(c + 1) * 128],
                                            src_y[:, c, kc * 128:(kc + 1) * 128], ident)
                    nc.vector.tensor_copy(out=ycT[:, kc, :], in_=ps)
                return ycT

            def bcast_vec(vec_ap, width):
                t = bcast.tile([128, width], F32, tag="bc")
                src = bass.AP(tensor=vec_ap.tensor, offset=vec_ap.offset,
                              ap=[[0, 128]] + [list(x) for x in vec_ap.ap])
                nc.sync.dma_start(out=t, in_=src)
                return t

            def attention(lidx, ycT_cur, ybT, kvT_w, bkv_w, wq_w, bq_w, wo_w, bo_w,
                          kv_src, kv_len, use_mask, gj, y_res):
                """Generic attention. kv_src: [128, 4, kv_len] feature-major y/enc.
                Returns new y_chunk tile (post-residual-LN, ln index gj)."""
                # --- weights ---
                wkvh = []
                for h2 in range(2):  # halves of [D, 2D] weight
                    wt = wproj.tile([128, 4, D], F32, tag="wp")
                    nc.sync.dma_start(
                        out=wt, in_=kvT_w[lidx].rearrange("(kc p) m -> p kc m", p=128)
                        [:, :, h2 * D:(h2 + 1) * D])
                    wkvh.append(wt)
                wqt = wproj.tile([128, 4, D], F32, tag="wp")
                nc.sync.dma_start(out=wqt, in_=wq_w[lidx].rearrange("(kc p) m -> p kc m", p=128))
                wot = wproj.tile([128, 4, D], F32, tag="wp")
                nc.sync.dma_start(out=wot, in_=wo_w[lidx].rearrange("(kc p) m -> p kc m", p=128))
                bkv_sb = small.tile([128, 8], F32, tag="bkv")
                nc.sync.dma_start(out=bkv_sb, in_=bkv_w[lidx].rearrange("(mc p) -> p mc", p=128))
                bq_sb = small.tile([128, 4], F32, tag="bq")
                nc.sync.dma_start(out=bq_sb, in_=bq_w[lidx].rearrange("(mc p) -> p mc", p=128))

                nkc = kv_len // 512  # 512-wide rhs chunks
                # --- K/V projections (feature-major) ---
                kT = kTp.tile([128, 4, kv_len], F32, tag="kT")
                vT = big2.tile([128, 4, kv_len], F32, tag="b2")
                for mc in range(8):
                    dst = kT if mc < 4 else vT
                    wt = wkvh[mc // 4]
                    mloc = mc % 4
                    ps = psA.tile([128, kv_len], F32, tag="a")
                    for f in range(nkc):
                        for kc in range(4):
                            nc.tensor.matmul(
                                ps[:, f * 512:(f + 1) * 512],
                                wt[:, kc, mloc * 128:(mloc + 1) * 128],
                                kv_src[:, kc, f * 512:(f + 1) * 512],
                                start=(kc == 0), stop=(kc == 3))
                    nc.scalar.activation(out=dst[:, mloc, :], in_=ps, func=AF.Identity,
                                         bias=bkv_sb[:, mc:mc + 1], scale=1.0)
                # --- V -> token-major ---
                v_tok = vtokp.tile([128, 8, D], F32, tag="vtok")
                for tk in range(kv_len // 128):
                    ps = psB.tile([128, 512], F32, tag="b")
                    for mv in range(4):
                        nc.tensor.transpose(ps[:, mv * 128:(mv + 1) * 128],
                                            vT[:, mv, tk * 128:(tk + 1) * 128], ident)
                    nc.vector.tensor_copy(out=v_tok[:, tk, :], in_=ps)
                # --- Q (scaled by 1/8, feature-major) ---
                qT = qavp.tile([128, 4, NCHUNK], F32, tag="qav")
                for mc in range(4):
                    ps = psC.tile([128, 256], F32, tag="c")
                    for kc in range(4):
                        nc.tensor.matmul(ps, wqt[:, kc, mc * 128:(mc + 1) * 128],
                                         ycT_cur[:, kc, :], start=(kc == 0), stop=(kc == 3))
                    nc.scalar.activation(out=qT[:, mc, :], in_=ps, func=AF.Identity,
                                         bias=bq_sb[:, mc:mc + 1], scale=0.125)
                # --- per head/chunk: scores, softmax, transpose, A@V ---
                avT = qavp.tile([128, 4, NCHUNK], F32, tag="qav")
                for c in range(2):
                    for h in range(H):
                        mc, po = h // 2, (h % 2) * 64
                        ps = psA.tile([128, kv_len], F32, tag="a")
                        for f in range(nkc):
                            nc.tensor.matmul(
                                ps[:, f * 512:(f + 1) * 512],
                                qT[po:po + 64, mc, c * 128:(c + 1) * 128],
                                kT[po:po + 64, mc, f * 512:(f + 1) * 512],
                                start=True, stop=True)
                        if use_mask:
                            s_t = smax.tile([128, kv_len], F32, tag="sm")
                            nc.vector.tensor_add(out=s_t, in0=ps, in1=mask_t[:, c, :])
                            src = s_t
                        else:
                            src = ps
                        rmax = small.tile([128, 1], F32, tag="rmax")
                        nc.vector.reduce_max(rmax, src, axis=mybir.AxisListType.X)
                        nmax = small.tile([128, 1], F32, tag="nmax")
                        nc.scalar.mul(out=nmax, in_=rmax, mul=-1.0)
                        p_t = smax.tile([128, kv_len], F32, tag="sm")
                        rsum = small.tile([128, 1], F32, tag="rsum")
                        nc.scalar.activation(out=p_t, in_=src, func=AF.Exp,
                                             bias=nmax, scale=1.0, accum_out=rsum)
                        rinv = small.tile([128, 1], F32, tag="rinv")
                        nc.vector.reciprocal(out=rinv, in_=rsum)
                        nc.vector.tensor_scalar_mul(p_t, p_t, rinv)
                        # transpose attn -> [sk, tq]
                        attnT = smax.tile([128, kv_len // 128, 128], F32, tag="sm")
                        for g4 in range(kv_len // 512):
                            ps2 = psB.tile([128, 512], F32, tag="b")
                            for i in range(4):
                                sk = g4 * 4 + i
                                nc.tensor.transpose(ps2[:, i * 128:(i + 1) * 128],
                                                    p_t[:, sk * 128:(sk + 1) * 128], ident)
                            nc.vector.tensor_copy(out=attnT[:, g4 * 4:(g4 + 1) * 4, :], in_=ps2)
                        # A @ V -> avT[dh, tq]
                        psv = psC.tile([64, 128], F32, tag="c")
                        nsk = kv_len // 128
                        for sk in range(nsk):
                            nc.tensor.matmul(psv, v_tok[:, sk, h * 64:(h + 1) * 64],
                                             attnT[:, sk, :],
                                             start=(sk == 0), stop=(sk == nsk - 1))
                        nc.vector.tensor_copy(out=avT[po:po + 64, mc, c * 128:(c + 1) * 128],
                                              in_=psv)
                # --- out proj + bias + residual + LN ---
                bo_bc = bcast_vec(bo_w[lidx], D)
                g_bc = bcast_vec(ln_g[lidx, gj], D)
                b_bc = bcast_vec(ln_b[lidx, gj], D)
                y_new = ypool.tile([128, 2, D], F32, tag="y")
                for c in range(2):
                    ps = psB.tile([128, 512], F32, tag="b")
                    for kc in range(4):
                        nc.tensor.matmul(ps, avT[:, kc, c * 128:(c + 1) * 128],
                                         wot[:, kc, :], start=(kc == 0), stop=(kc == 3))
                    t = smax.tile([128, D], F32, tag="res")
                    nc.vector.tensor_add(out=t, in0=ps, in1=bo_bc)
                    nc.vector.tensor_add(out=t, in0=t, in1=y_res[:, c, :])
                    _ln(nc, pools, y_new[:, c, :], t, g_bc, b_bc, small)
                return y_new

            # ================= layers =================
            for lidx in range(n_layers):
                ycT = transpose_chunk(y_chunk)
                # --- gather y over the batch group ---
                if lidx == 0:
                    ybT = ybTp.tile([128, 4, TB], F32, tag="ybT")
                    nc.sync.dma_start(out=ybT,
                                      in_=y0T_b.rearrange("(kc p) t -> p kc t", p=128))
                else:
                    ag_in = dramp.tile([D, NCHUNK], F32, tag="agin")
                    nc.sync.dma_start(out=ag_in.rearrange("(kc p) t -> p kc t", p=128),
                                      in_=ycT)
                    ag_out = dramp.tile([4 * D, NCHUNK], F32, tag="agout")
                    nc.gpsimd.collective_compute(
                        "AllGather", ALU.bypass, replica_groups=GROUPS,
                        ins=[ag_in[:]], outs=[ag_out[:]])
                    ybT = ybTp.tile([128, 4, TB], F32, tag="ybT")
                    ag_v = ag_out.rearrange("(r c p) t -> r p c t", r=4, c=4, p=128)
                    for r in range(4):
                        nc.sync.dma_start(out=ybT[:, :, r * NCHUNK:(r + 1) * NCHUNK],
                                          in_=ag_v[r])

                # --- self-attention + LN0 ---
                y1 = attention(lidx, ycT, ybT, wkv_sa, bkv_sa, wq_sa, bq_sa,
                               wo_sa, bo_sa, ybT, TB, True, 0, y_chunk)
                if debug:
                    nc.sync.dma_start(out=dbg[lidx, 0].rearrange("(c p) d -> p c d", p=128),
                                      in_=y1)
                # --- cross-attention + LN1 ---
                y1cT = transpose_chunk(y1)
                y2 = attention(lidx, y1cT, None, wkv_ca, bkv_ca, wq_ca, bq_ca,
                               wo_ca, bo_ca, encT, S, False, 1, y1)
                if debug:
                    nc.sync.dma_start(out=dbg[lidx, 1].rearrange("(c p) d -> p c d", p=128),
                                      in_=y2)
                y2cT = transpose_chunk(y2)

                # --- router ---
                rw_t = small.tile([128, 4, E], F32, tag="rw")
                nc.sync.dma_start(out=rw_t, in_=rw[lidx].rearrange("(kc p) e -> p kc e", p=128))
                rb_ap = rb[lidx]
                rb_bc = small.tile([128, E], F32, tag="rbbc")
                nc.sync.dma_start(out=rb_bc, in_=bass.AP(
                    tensor=rb_ap.tensor, offset=rb_ap.offset,
                    ap=[[0, 128]] + [list(x) for x in rb_ap.ap]))
                probs = small.tile([128, 2, E], F32, tag="probs")
                gates = small.tile([128, 2, E], F32, tag="gates")
                for c in range(2):
                    ps = psC.tile([128, E], F32, tag="c")
                    for kc in range(4):
                        nc.tensor.matmul(ps, y2cT[:, kc, c * 128:(c + 1) * 128], rw_t[:, kc, :],
                                         start=(kc == 0), stop=(kc == 3))
                    logit = small.tile([128, E], F32, tag="logit")
                    nc.vector.tensor_add(out=logit, in0=ps, in1=rb_bc)
                    rmax = small.tile([128, 1], F32, tag="rmax")
                    nc.vector.reduce_max(rmax, logit, axis=mybir.AxisListType.X)
                    nmax = small.tile([128, 1], F32, tag="nmax")
                    nc.scalar.mul(out=nmax, in_=rmax, mul=-1.0)
                    rsum = small.tile([128, 1], F32, tag="rsum")
                    nc.scalar.activation(out=probs[:, c, :], in_=logit, func=AF.Exp,
                                         bias=nmax, scale=1.0, accum_out=rsum)
                    rinv = small.tile([128, 1], F32, tag="rinv")
                    nc.vector.reciprocal(out=rinv, in_=rsum)
                    nc.vector.tensor_scalar_mul(probs[:, c, :], probs[:, c, :], rinv)
                    # top-2 gates
                    m8 = small.tile([128, 8], F32, tag="m8")
                    nc.vector.max(out=m8, in_=probs[:, c, :])
                    vs = small.tile([128, 1], F32, tag="vs")
                    nc.vector.tensor_add(out=vs, in0=m8[:, 0:1], in1=m8[:, 1:2])
                    gv = small.tile([128, 1], F32, tag="gv")
                    nc.vector.reciprocal(out=gv, in_=vs)
                    km = small.tile([128, E], F32, tag="km")
                    nc.vector.tensor_scalar(out=km, in0=probs[:, c, :], scalar1=m8[:, 1:2],
                                            scalar2=None, op0=ALU.is_ge)
                    nc.vector.tensor_mul(out=gates[:, c, :], in0=probs[:, c, :], in1=km)
                    nc.vector.tensor_scalar_mul(gates[:, c, :], gates[:, c, :], gv)
                nc.sync.dma_start(out=probs_out[lidx].rearrange("(c p) e -> p c e", p=128),
                                  in_=probs)

                # --- dense MoE ---
                moe_acc = maccp.tile([128, 2, D], F32, tag="macc")
                for e in range(E):
                    b1_sb = small.tile([128, 16], F32, tag="b1")
                    nc.sync.dma_start(out=b1_sb, in_=eb1[lidx, e].rearrange("(fc p) -> p fc", p=128))
                    b2_bc = bcast_vec(eb2[lidx, e], D)
                    hT = big2.tile([128, 16, NCHUNK], F32, tag="b2")
                    for fq in range(4):  # quarters of w1 columns
                        w1q = stream.tile([128, 4, 512], F32, tag="w1")
                        nc.sync.dma_start(
                            out=w1q,
                            in_=ew1[lidx, e].rearrange("(kc p) f -> p kc f", p=128)
                            [:, :, fq * 512:(fq + 1) * 512])
                        for fm in range(4):
                            fc = fq * 4 + fm
                            ps = psC.tile([128, NCHUNK], F32, tag="c")
                            for kc in range(4):
                                nc.tensor.matmul(ps, w1q[:, kc, fm * 128:(fm + 1) * 128],
                                                 y2cT[:, kc, :], start=(kc == 0), stop=(kc == 3))
                            nc.scalar.activation(out=hT[:, fc, :], in_=ps, func=AF.Relu,
                                                 bias=b1_sb[:, fc:fc + 1], scale=1.0)
                    psE = [psB.tile([128, 512], F32, tag="b", name=f"psE{_c}") for _c in range(2)]
                    for qq in range(4):  # quarters of w2 rows
                        w2q = stream.tile([128, 4, 512], F32, tag="w1")
                        nc.sync.dma_start(
                            out=w2q,
                            in_=ew2[lidx, e].rearrange("(q c p) d -> q p c d", q=4, p=128)[qq])
                        for c in range(2):
                            for cc in range(4):
                                fc = qq * 4 + cc
                                nc.tensor.matmul(psE[c], hT[:, fc, c * 128:(c + 1) * 128],
                                                 w2q[:, cc, :],
                                                 start=(qq == 0 and cc == 0),
                                                 stop=(qq == 3 and cc == 3))
                    for c in range(2):
                        t = smax.tile([128, D], F32, tag="res")
                        nc.vector.tensor_add(out=t, in0=psE[c], in1=b2_bc)
                        if e == 0:
                            nc.vector.tensor_scalar(out=moe_acc[:, c, :], in0=t,
                                                    scalar1=gates[:, c, e:e + 1],
                                                    scalar2=None, op0=ALU.mult)
                        else:
                            t2 = smax.tile([128, D], F32, tag="res")
                            nc.vector.tensor_scalar(out=t2, in0=t,
                                                    scalar1=gates[:, c, e:e + 1],
                                                    scalar2=None, op0=ALU.mult)
                            nc.vector.tensor_add(out=moe_acc[:, c, :],
                                                 in0=moe_acc[:, c, :], in1=t2)
                # --- residual + LN2 ---
                g_bc = bcast_vec(ln_g[lidx, 2], D)
                b_bc = bcast_vec(ln_b[lidx, 2], D)
                y3 = ypool.tile([128, 2, D], F32, tag="y")
                for c in range(2):
                    t = smax.tile([128, D], F32, tag="res")
                    nc.vector.tensor_add(out=t, in0=moe_acc[:, c, :], in1=y2[:, c, :])
                    _ln(nc, pools, y3[:, c, :], t, g_bc, b_bc, small)
                if debug:
                    nc.sync.dma_start(out=dbg[lidx, 2].rearrange("(c p) d -> p c d", p=128),
                                      in_=y3)
                y_chunk = y3

            nc.sync.dma_start(out=out_y.rearrange("(c p) d -> p c d", p=128), in_=y_chunk)

    nc.compile()
    return nc


def host_prep(inputs):
    """Build per-core in_maps from full inputs."""
    f32 = lambda x: np.ascontiguousarray(np.asarray(x), dtype=np.float32)
    ids = np.asarray(inputs["decoder_input_ids"])
    emb = f32(inputs["embed_table"])
    pos = f32(inputs["pos_table"])
    y0 = emb[ids] + pos[:T][None]                   # [B, T, D] fp32
    enc = f32(inputs["enc_out"])
    am = np.asarray(inputs["attention_mask"]) != 0  # [B, T]

    sa_w = f32(inputs["sa_w"]); sa_b = f32(inputs["sa_b"])
    ca_w = f32(inputs["ca_w"]); ca_b = f32(inputs["ca_b"])

    def pack(w, b):
        return dict(
            wkv=np.ascontiguousarray(np.concatenate([w[:, 1], w[:, 2]], axis=2)),
            bkv=np.ascontiguousarray(np.concatenate([b[:, 1], b[:, 2]], axis=1)),
            wq=np.ascontiguousarray(w[:, 0]), bq=np.ascontiguousarray(b[:, 0] * 0.125),
            wo=np.ascontiguousarray(w[:, 3]), bo=np.ascontiguousarray(b[:, 3]))

    sa = pack(sa_w, sa_b)
    ca = pack(ca_w, ca_b)
    shared = {
        "wkv_sa": sa["wkv"], "bkv_sa": sa["bkv"], "wq_sa": sa["wq"], "bq_sa": sa["bq"],
        "wo_sa": sa["wo"], "bo_sa": sa["bo"],
        "wkv_ca": ca["wkv"], "bkv_ca": ca["bkv"], "wq_ca": ca["wq"], "bq_ca": ca["bq"],
        "wo_ca": ca["wo"], "bo_ca": ca["bo"],
        "ln_g": f32(inputs["ln_g"]), "ln_b": f32(inputs["ln_b"]),
        "rw": f32(inputs["router_w"]), "rb": f32(inputs["router_b"]),
        "ew1": f32(inputs["exp_w1"]), "eb1": f32(inputs["exp_b1"]),
        "ew2": f32(inputs["exp_w2"]), "eb2": f32(inputs["exp_b2"]),
    }

    in_maps = []
    causal_row = np.arange(TB)[None, :]
    for c in range(N_CORES):
        b, q = c // 4, c % 4
        rows = np.arange(q * NCHUNK, (q + 1) * NCHUNK)[:, None]
        ok = (causal_row <= rows) & am[b][None, :]
        mask = np.where(ok, np.float32(0.0), np.float32(-1e9)).astype(np.float32)
        m = dict(shared)
        m["y0_chunk"] = np.ascontiguousarray(y0[b, q * NCHUNK:(q + 1) * NCHUNK])
        m["y0T_b"] = np.ascontiguousarray(y0[b].T)
        m["mask"] = mask
        m["encT_b"] = np.ascontiguousarray(enc[b].T)
        in_maps.append(m)
    return in_maps


def kernel(**inputs):
    if "nc" not in _cache:
        _cache["nc"] = build()
    nc = _cache["nc"]
    in_maps = host_prep(inputs)
    res = run_bass_kernel_spmd(nc, in_maps, list(range(N_CORES)))
    _cache["last_results"] = res

    y = np.empty((B, T, D), np.float32)
    probs = np.empty((L, B * T, E), np.float32)
    for c in range(N_CORES):
        b, q = c // 4, c % 4
        y[b, q * NCHUNK:(q + 1) * NCHUNK] = res.results[c]["out_y"]
        probs[:, b * T + q * NCHUNK:b * T + (q + 1) * NCHUNK] = res.results[c]["probs_out"]

    # load-balancing aux loss from device-produced router probs
    lb_total = np.float32(0.0)
    for lidx in range(L):
        p = probs[lidx]                              # [B*T, E]
        idx = np.argsort(-p, axis=-1, kind="stable")[:, :K]
        kmask = np.zeros_like(p)
        np.put_along_axis(kmask, idx, 1.0, axis=-1)
        f = kmask.mean(0, dtype=np.float32)
        pm = p.mean(0, dtype=np.float32)
        lb_total = np.float32(lb_total + np.float32(E) * np.float32(np.sum(f * pm, dtype=np.float32)))
    return y, lb_total


# revision 17
# speedup vs baseline: 1.2600x; 1.2600x over previous
"""Trainium2 Bass kernel for nn_Decoder (moe_routing): 4-layer decoder with
self-attn + cross-attn + top-2-of-8 MoE per layer.

Distribution (8 NeuronCores): token-sharded. Core c owns 256 tokens
(cores 0-3 = batch 0, cores 4-7 = batch 1, chunk q = c % 4). Cross-attention
and the MoE are fully token-local. Self-attention needs full-batch K/V, so
each layer starts with one grouped AllGather of y over the 4 cores of each
batch; K/V projections are recomputed per core (causality handled by an
additive mask, so all cores run an identical program).

The MoE is computed densely (all 8 experts on the core's 256 tokens) in fp32
and combined with the renormalized top-2 gates, exactly mirroring the
reference math. Expert weights stream from HBM (64 MB/layer/core).

All matmuls are plain fp32: the router's top-2 margins go down to ~1e-5, so
tf32-like fast modes (fp32r/bf16) flip expert selection and are not usable.

Host side: embedding + positional lookup, input layout prep (transposes,
concatenated K/V weights, mask construction), final output assembly and the
load-balancing aux loss (computed from the device-produced router probs).
"""
import os
import numpy as np

import concourse.bass as bass
import concourse.mybir as mybir
import concourse.tile as tile
from concourse import bacc
from concourse.bass_utils import run_bass_kernel_spmd
from concourse.masks import make_identity

F32 = mybir.dt.float32
AF = mybir.ActivationFunctionType
ALU = mybir.AluOpType

B, T, S = 2, 1024, 1024
D, H, L = 512, 8, 4
E, K, FF = 8, 2, 2048
DH = D // H
EPS = 1e-5
NCHUNK = 256            # tokens per core
TB = 1024               # tokens per batch
N_CORES = 8
GROUPS = [[0, 1, 2, 3], [4, 5, 6, 7]]

DEBUG = bool(int(os.environ.get("KERNEL_DEBUG", "0")))
N_LAYERS = int(os.environ.get("KERNEL_LAYERS", str(L)))

_cache = {}


def _ln(nc, pools, out_ap, in_ap, g_bcast, b_bcast, stats_pool):
    """LayerNorm over free dim (512) of [128, 512] in_ap -> out_ap."""
    st = stats_pool.tile([128, 6], F32, tag="st")
    nc.vector.bn_stats(out=st, in_=in_ap)
    mv = stats_pool.tile([128, 2], F32, tag="mv")
    nc.vector.bn_aggr(out=mv, in_=st)
    sq = stats_pool.tile([128, 1], F32, tag="sq")
    nc.scalar.activation(out=sq, in_=mv[:, 1:2], func=AF.Sqrt,
                         bias=pools["eps"][:, 0:1], scale=1.0)
    rstd = stats_pool.tile([128, 1], F32, tag="rstd")
    nc.vector.reciprocal(out=rstd, in_=sq)
    nc.vector.tensor_scalar(out=in_ap, in0=in_ap, scalar1=mv[:, 0:1], scalar2=rstd,
                            op0=ALU.subtract, op1=ALU.mult)
    nc.vector.tensor_mul(out=in_ap, in0=in_ap, in1=g_bcast)
    nc.vector.tensor_add(out=out_ap, in0=in_ap, in1=b_bcast)


def build(n_layers=N_LAYERS, debug=DEBUG):
    nc = bacc.Bacc(None, target_bir_lowering=False, num_devices=N_CORES)

    # ---- I/O ----
    inp = {}
    def din(name, shape, dtype=F32):
        inp[name] = nc.dram_tensor(name, list(shape), dtype, kind="ExternalInput")
        return inp[name]

    y0_chunk = din("y0_chunk", [NCHUNK, D])
    y0T_b = din("y0T_b", [D, TB])
    mask_in = din("mask", [NCHUNK, TB])
    encT_in = din("encT_b", [D, S])
    wkv_sa = din("wkv_sa", [L, D, 2 * D]); bkv_sa = din("bkv_sa", [L, 2 * D])
    wq_sa = din("wq_sa", [L, D, D]); bq_sa = din("bq_sa", [L, D])
    wo_sa = din("wo_sa", [L, D, D]); bo_sa = din("bo_sa", [L, D])
    wkv_ca = din("wkv_ca", [L, D, 2 * D]); bkv_ca = din("bkv_ca", [L, 2 * D])
    wq_ca = din("wq_ca", [L, D, D]); bq_ca = din("bq_ca", [L, D])
    wo_ca = din("wo_ca", [L, D, D]); bo_ca = din("bo_ca", [L, D])
    ln_g = din("ln_g", [L, 3, D]); ln_b = din("ln_b", [L, 3, D])
    rw = din("rw", [L, D, E]); rb = din("rb", [L, E])
    iota_in = din("iota_f", [128, 128])
    umat_in = din("umat", [NCHUNK, NCHUNK])
    ew1 = din("ew1", [L, E, D, FF]); eb1 = din("eb1", [L, E, FF])
    ew2 = din("ew2", [L, E, FF, D]); eb2 = din("eb2", [L, E, D])

    out_y = nc.dram_tensor("out_y", [NCHUNK, D], F32, kind="ExternalOutput")
    probs_out = nc.dram_tensor("probs_out", [L, NCHUNK, E], F32, kind="ExternalOutput")
    if debug:
        dbg = nc.dram_tensor("dbg", [L, 3, NCHUNK, D], F32, kind="ExternalOutput")
        dbg_cs = nc.dram_tensor("dbg_cs", [8, NCHUNK], F32, kind="ExternalOutput")
        dbg_slot = nc.dram_tensor("dbg_slot", [128, 2, E], F32, kind="ExternalOutput")
        dbg_ysel = nc.dram_tensor("dbg_ysel", [128, 4, 128], F32, kind="ExternalOutput")
        dbg_eosel = nc.dram_tensor("dbg_eosel", [128, 512], F32, kind="ExternalOutput")
        dbg_moe = nc.dram_tensor("dbg_moe", [2, 128, 512], F32, kind="ExternalOutput")
        dbg_pt = nc.dram_tensor("dbg_pt", [128, 128], F32, kind="ExternalOutput")

    with tile.TileContext(nc) as tc:
        import contextlib
        ctx = contextlib.ExitStack()
        with ctx:
            const = ctx.enter_context(tc.tile_pool(name="const", bufs=1))
            ypool = ctx.enter_context(tc.tile_pool(name="ypool", bufs=2))
            ycTp = ctx.enter_context(tc.tile_pool(name="ycTp", bufs=2))
            ybTp = ctx.enter_context(tc.tile_pool(name="ybTp", bufs=1))
            kTp = ctx.enter_context(tc.tile_pool(name="kTp", bufs=1))
            vtokp = ctx.enter_context(tc.tile_pool(name="vtokp", bufs=1))
            wproj = ctx.enter_context(tc.tile_pool(name="wproj", bufs=2))
            smax = ctx.enter_context(tc.tile_pool(name="smax", bufs=3))
            qavp = ctx.enter_context(tc.tile_pool(name="qavp", bufs=2))
            big2 = ctx.enter_context(tc.tile_pool(name="big2", bufs=1))
            stream = ctx.enter_context(tc.tile_pool(name="stream", bufs=2))
            moep = ctx.enter_context(tc.tile_pool(name="moep", bufs=2))
            atp = ctx.enter_context(tc.tile_pool(name="atp", bufs=2))
            bcast = ctx.enter_context(tc.tile_pool(name="bcast", bufs=3))
            small = ctx.enter_context(tc.tile_pool(name="small", bufs=4))
            dramp = ctx.enter_context(tc.tile_pool(name="dramp", bufs=2, space="DRAM"))
            psA = ctx.enter_context(tc.tile_pool(name="psA", bufs=2, space="PSUM"))
            psB = ctx.enter_context(tc.tile_pool(name="psB", bufs=2, space="PSUM"))
            psC = ctx.enter_context(tc.tile_pool(name="psC", bufs=2, space="PSUM"))

            pools = {}

            ident = const.tile([128, 128], F32)
            make_identity(nc, ident)
            eps_t = const.tile([128, 1], F32)
            nc.vector.memset(eps_t, EPS)
            pools["eps"] = eps_t

            mask_t = const.tile([128, 2, TB], F32)
            nc.sync.dma_start(out=mask_t, in_=mask_in.rearrange("(c p) t -> p c t", p=128))
            encT = const.tile([128, 4, S], F32)
            nc.sync.dma_start(out=encT, in_=encT_in.rearrange("(kc p) t -> p kc t", p=128))

            iota_f = const.tile([128, 128], F32)
            nc.sync.dma_start(out=iota_f, in_=iota_in[:])
            umat_t = const.tile([128, 2, NCHUNK], F32)
            nc.sync.dma_start(out=umat_t, in_=umat_in.rearrange("(c p) t -> p c t", p=128))

            y_chunk = ypool.tile([128, 2, D], F32, tag="y")
            nc.sync.dma_start(out=y_chunk, in_=y0_chunk.rearrange("(c p) d -> p c d", p=128))

            def transpose_chunk(src_y):
                """[128, 2, 512] token-major -> ycT [128, 4, 256] feature-major."""
                ycT = ycTp.tile([128, 4, NCHUNK], F32, tag="ycT")
                for kc in range(4):
                    ps = psC.tile([128, 256], F32, tag="c")
                    for c in range(2):
                        nc.tensor.transpose(ps[:, c * 128:(c + 1) * 128],
                                            src_y[:, c, kc * 128:(kc + 1) * 128], ident)
                    nc.vector.tensor_copy(out=ycT[:, kc, :], in_=ps)
                return ycT

            def bcast_vec(vec_ap, width):
                t = bcast.tile([128, width], F32, tag="bc")
                src = bass.AP(tensor=vec_ap.tensor, offset=vec_ap.offset,
                              ap=[[0, 128]] + [list(x) for x in vec_ap.ap])
                nc.sync.dma_start(out=t, in_=src)
                return t

            def attention(lidx, ycT_cur, ybT, kvT_w, bkv_w, wq_w, bq_w, wo_w, bo_w,
                          kv_src, kv_len, use_mask, gj, y_res):
                """Generic attention. kv_src: [128, 4, kv_len] feature-major y/enc.
                Returns new y_chunk tile (post-residual-LN, ln index gj)."""
                # --- weights ---
                wkvh = []
                for h2 in range(2):  # halves of [D, 2D] weight
                    wt = wproj.tile([128, 4, D], F32, tag="wp")
                    nc.sync.dma_start(
                        out=wt, in_=kvT_w[lidx].rearrange("(kc p) m -> p kc m", p=128)
                        [:, :, h2 * D:(h2 + 1) * D])
                    wkvh.append(wt)
                wqt = wproj.tile([128, 4, D], F32, tag="wp")
                nc.sync.dma_start(out=wqt, in_=wq_w[lidx].rearrange("(kc p) m -> p kc m", p=128))
                wot = wproj.tile([128, 4, D], F32, tag="wp")
                nc.sync.dma_start(out=wot, in_=wo_w[lidx].rearrange("(kc p) m -> p kc m", p=128))
                bkv_sb = small.tile([128, 8], F32, tag="bkv")
                nc.sync.dma_start(out=bkv_sb, in_=bkv_w[lidx].rearrange("(mc p) -> p mc", p=128))
                bq_sb = small.tile([128, 4], F32, tag="bq")
                nc.sync.dma_start(out=bq_sb, in_=bq_w[lidx].rearrange("(mc p) -> p mc", p=128))

                nkc = kv_len // 512  # 512-wide rhs chunks
                # --- K/V projections (feature-major) ---
                kT = kTp.tile([128, 4, kv_len], F32, tag="kT")
                vT = big2.tile([128, 4, kv_len], F32, tag="b2")
                for mc in range(8):
                    dst = kT if mc < 4 else vT
                    wt = wkvh[mc // 4]
                    mloc = mc % 4
                    ps = psA.tile([128, kv_len], F32, tag="a")
                    for f in range(nkc):
                        for kc in range(4):
                            nc.tensor.matmul(
                                ps[:, f * 512:(f + 1) * 512],
                                wt[:, kc, mloc * 128:(mloc + 1) * 128],
                                kv_src[:, kc, f * 512:(f + 1) * 512],
                                start=(kc == 0), stop=(kc == 3))
                    nc.scalar.activation(out=dst[:, mloc, :], in_=ps, func=AF.Identity,
                                         bias=bkv_sb[:, mc:mc + 1], scale=1.0)
                # --- V -> token-major ---
                v_tok = vtokp.tile([128, 8, D], F32, tag="vtok")
                for tk in range(kv_len // 128):
                    ps = psB.tile([128, 512], F32, tag="b")
                    for mv in range(4):
                        nc.tensor.transpose(ps[:, mv * 128:(mv + 1) * 128],
                                            vT[:, mv, tk * 128:(tk + 1) * 128], ident)
                    nc.vector.tensor_copy(out=v_tok[:, tk, :], in_=ps)
                # --- Q (scaled by 1/8, feature-major) ---
                qT = qavp.tile([128, 4, NCHUNK], F32, tag="qav")
                for mc in range(4):
                    ps = psC.tile([128, 256], F32, tag="c")
                    for kc in range(4):
                        nc.tensor.matmul(ps, wqt[:, kc, mc * 128:(mc + 1) * 128],
                                         ycT_cur[:, kc, :], start=(kc == 0), stop=(kc == 3))
                    nc.scalar.activation(out=qT[:, mc, :], in_=ps, func=AF.Identity,
                                         bias=bq_sb[:, mc:mc + 1], scale=0.125)
                # --- per head/chunk: scores, softmax, transpose, A@V ---
                avT = qavp.tile([128, 4, NCHUNK], F32, tag="qav")
                for h in range(H):
                    mc, po = h // 2, (h % 2) * 64
                    attnT = atp.tile([128, kv_len // 128, NCHUNK], F32, tag="at")
                    for c in range(2):
                        ps = psA.tile([128, kv_len], F32, tag="a")
                        for f in range(nkc):
                            nc.tensor.matmul(
                                ps[:, f * 512:(f + 1) * 512],
                                qT[po:po + 64, mc, c * 128:(c + 1) * 128],
                                kT[po:po + 64, mc, f * 512:(f + 1) * 512],
                                start=True, stop=True)
                        if use_mask:
                            s_t = smax.tile([128, kv_len], F32, tag="sm")
                            nc.vector.tensor_add(out=s_t, in0=ps, in1=mask_t[:, c, :])
                            src = s_t
                        else:
                            src = ps
                        rmax = small.tile([128, 1], F32, tag="rmax")
                        nc.vector.reduce_max(rmax, src, axis=mybir.AxisListType.X)
                        nmax = small.tile([128, 1], F32, tag="nmax")
                        nc.scalar.mul(out=nmax, in_=rmax, mul=-1.0)
                        p_t = smax.tile([128, kv_len], F32, tag="sm")
                        rsum = small.tile([128, 1], F32, tag="rsum")
                        nc.scalar.activation(out=p_t, in_=src, func=AF.Exp,
                                             bias=nmax, scale=1.0, accum_out=rsum)
                        rinv = small.tile([128, 1], F32, tag="rinv")
                        nc.vector.reciprocal(out=rinv, in_=rsum)
                        nc.vector.tensor_scalar_mul(p_t, p_t, rinv)
                        # transpose attn -> [sk, tq] into both-chunk attnT
                        for g4 in range(kv_len // 512):
                            ps2 = psB.tile([128, 512], F32, tag="b")
                            for i in range(4):
                                sk = g4 * 4 + i
                                nc.tensor.transpose(ps2[:, i * 128:(i + 1) * 128],
                                                    p_t[:, sk * 128:(sk + 1) * 128], ident)
                            nc.vector.tensor_copy(
                                out=attnT[:, g4 * 4:(g4 + 1) * 4, c * 128:(c + 1) * 128],
                                in_=ps2.rearrange("p (a t) -> p a t", a=4))
                    # A @ V -> avT[dh, tq] (both chunks at once)
                    psv = psC.tile([64, NCHUNK], F32, tag="c")
                    nsk = kv_len // 128
                    for sk in range(nsk):
                        nc.tensor.matmul(psv, v_tok[:, sk, h * 64:(h + 1) * 64],
                                         attnT[:, sk, :],
                                         start=(sk == 0), stop=(sk == nsk - 1))
                    nc.vector.tensor_copy(out=avT[po:po + 64, mc, :], in_=psv)
                # --- out proj + bias + residual + LN ---
                bo_bc = bcast_vec(bo_w[lidx], D)
                g_bc = bcast_vec(ln_g[lidx, gj], D)
                b_bc = bcast_vec(ln_b[lidx, gj], D)
                y_new = ypool.tile([128, 2, D], F32, tag="y")
                for c in range(2):
                    ps = psB.tile([128, 512], F32, tag="b")
                    for kc in range(4):
                        nc.tensor.matmul(ps, avT[:, kc, c * 128:(c + 1) * 128],
                                         wot[:, kc, :], start=(kc == 0), stop=(kc == 3))
                    t = smax.tile([128, D], F32, tag="res")
                    nc.vector.tensor_add(out=t, in0=ps, in1=bo_bc)
                    nc.vector.tensor_add(out=t, in0=t, in1=y_res[:, c, :])
                    _ln(nc, pools, y_new[:, c, :], t, g_bc, b_bc, small)
                return y_new

            # ================= layers =================
            for lidx in range(n_layers):
                ycT = transpose_chunk(y_chunk)
                # --- gather y over the batch group ---
                if lidx == 0:
                    ybT = ybTp.tile([128, 4, TB], F32, tag="ybT")
                    nc.sync.dma_start(out=ybT,
                                      in_=y0T_b.rearrange("(kc p) t -> p kc t", p=128))
                else:
                    ag_in = dramp.tile([D, NCHUNK], F32, tag="agin")
                    nc.sync.dma_start(out=ag_in.rearrange("(kc p) t -> p kc t", p=128),
                                      in_=ycT)
                    ag_out = dramp.tile([4 * D, NCHUNK], F32, tag="agout")
                    nc.gpsimd.collective_compute(
                        "AllGather", ALU.bypass, replica_groups=GROUPS,
                        ins=[ag_in[:]], outs=[ag_out[:]])
                    ybT = ybTp.tile([128, 4, TB], F32, tag="ybT")
                    ag_v = ag_out.rearrange("(r c p) t -> r p c t", r=4, c=4, p=128)
                    for r in range(4):
                        nc.sync.dma_start(out=ybT[:, :, r * NCHUNK:(r + 1) * NCHUNK],
                                          in_=ag_v[r])

                # --- self-attention + LN0 ---
                y1 = attention(lidx, ycT, ybT, wkv_sa, bkv_sa, wq_sa, bq_sa,
                               wo_sa, bo_sa, ybT, TB, True, 0, y_chunk)
                if debug:
                    nc.sync.dma_start(out=dbg[lidx, 0].rearrange("(c p) d -> p c d", p=128),
                                      in_=y1)
                # --- cross-attention + LN1 ---
                y1cT = transpose_chunk(y1)
                y2 = attention(lidx, y1cT, None, wkv_ca, bkv_ca, wq_ca, bq_ca,
                               wo_ca, bo_ca, encT, S, False, 1, y1)
                if debug:
                    nc.sync.dma_start(out=dbg[lidx, 1].rearrange("(c p) d -> p c d", p=128),
                                      in_=y2)
                y2cT = transpose_chunk(y2)

                # --- router ---
                rw_t = small.tile([128, 4, E], F32, tag="rw")
                nc.sync.dma_start(out=rw_t, in_=rw[lidx].rearrange("(kc p) e -> p kc e", p=128))
                rb_ap = rb[lidx]
                rb_bc = small.tile([128, E], F32, tag="rbbc")
                nc.sync.dma_start(out=rb_bc, in_=bass.AP(
                    tensor=rb_ap.tensor, offset=rb_ap.offset,
                    ap=[[0, 128]] + [list(x) for x in rb_ap.ap]))
                probs = small.tile([128, 2, E], F32, tag="probs")
                gates = small.tile([128, 2, E], F32, tag="gates")
                m8_all = []
                for c in range(2):
                    ps = psC.tile([128, E], F32, tag="c")
                    for kc in range(4):
                        nc.tensor.matmul(ps, y2cT[:, kc, c * 128:(c + 1) * 128], rw_t[:, kc, :],
                                         start=(kc == 0), stop=(kc == 3))
                    logit = small.tile([128, E], F32, tag="logit")
                    nc.vector.tensor_add(out=logit, in0=ps, in1=rb_bc)
                    rmax = small.tile([128, 1], F32, tag="rmax")
                    nc.vector.reduce_max(rmax, logit, axis=mybir.AxisListType.X)
                    nmax = small.tile([128, 1], F32, tag="nmax")
                    nc.scalar.mul(out=nmax, in_=rmax, mul=-1.0)
                    rsum = small.tile([128, 1], F32, tag="rsum")
                    nc.scalar.activation(out=probs[:, c, :], in_=logit, func=AF.Exp,
                                         bias=nmax, scale=1.0, accum_out=rsum)
                    rinv = small.tile([128, 1], F32, tag="rinv")
                    nc.vector.reciprocal(out=rinv, in_=rsum)
                    nc.vector.tensor_scalar_mul(probs[:, c, :], probs[:, c, :], rinv)
                    # top-2 gates
                    m8 = small.tile([128, 8], F32, tag="m8", name=f"m8_{c}")
                    nc.vector.max(out=m8, in_=probs[:, c, :])
                    m8_all.append(m8[:, 1:2])
                    vs = small.tile([128, 1], F32, tag="vs")
                    nc.vector.tensor_add(out=vs, in0=m8[:, 0:1], in1=m8[:, 1:2])
                    gv = small.tile([128, 1], F32, tag="gv")
                    nc.vector.reciprocal(out=gv, in_=vs)
                    km = small.tile([128, E], F32, tag="km")
                    nc.vector.tensor_scalar(out=km, in0=probs[:, c, :], scalar1=m8[:, 1:2],
                                            scalar2=None, op0=ALU.is_ge)
                    nc.vector.tensor_mul(out=gates[:, c, :], in0=probs[:, c, :], in1=km)
                    nc.vector.tensor_scalar_mul(gates[:, c, :], gates[:, c, :], gv)
                nc.sync.dma_start(out=probs_out[lidx].rearrange("(c p) e -> p c e", p=128),
                                  in_=probs)

                # --- sparse top-2 MoE via permutation matmuls ---
                # exclusive prefix count of selected tokens per expert:
                # csT[e, t] = sum_{t'<t} kmask[t', e]   (umat[t',t] = 1 iff t'<t)
                km_t = moep.tile([128, 2, E], F32, tag="kmt")
                for c in range(2):
                    nc.vector.tensor_scalar(out=km_t[:, c, :], in0=probs[:, c, :],
                                            scalar1=m8_all[c], scalar2=None, op0=ALU.is_ge)
                ps_cs = psC.tile([8, NCHUNK], F32, tag="c")
                for c in range(2):
                    nc.tensor.matmul(ps_cs, km_t[:, c, :], umat_t[:, c, :],
                                     start=(c == 0), stop=(c == 1))
                csT = moep.tile([8, NCHUNK], F32, tag="csT")
                nc.vector.tensor_copy(out=csT, in_=ps_cs)
                if debug and lidx == 0:
                    nc.sync.dma_start(out=dbg_cs[:], in_=csT)
                slot_tok = moep.tile([128, 2, E], F32, tag="slot")
                for c in range(2):
                    pst = psC.tile([128, 128], F32, tag="c")
                    nc.tensor.transpose(pst[:, 0:8], csT[0:8, c * 128:(c + 1) * 128],
                                        ident[0:8, 0:8])
                    nc.vector.tensor_copy(out=slot_tok[:, c, :], in_=pst[:, 0:8])
                if debug and lidx == 0:
                    nc.sync.dma_start(out=dbg_slot[:], in_=slot_tok)
                # gates^T and b2 table for the Σ_e gate_e * b2_e term
                b2_all = moep.tile([8, D], F32, tag="b2a")
                nc.sync.dma_start(out=b2_all, in_=eb2[lidx])
                gT = moep.tile([8, NCHUNK], F32, tag="gT")
                for c in range(2):
                    psg = psC.tile([128, 128], F32, tag="c")
                    nc.tensor.transpose(psg[0:8, :], gates[:, c, :], ident)
                    nc.vector.tensor_copy(out=gT[:, c * 128:(c + 1) * 128], in_=psg[0:8, :])
                eo_acc = [psB.tile([128, 512], F32, tag="b", name=f"eoacc{_c}")
                          for _c in range(2)]
                for c in range(2):
                    nc.tensor.matmul(eo_acc[c], gT[0:8, c * 128:(c + 1) * 128], b2_all,
                                     start=True, stop=False)
                for e in range(E):
                    # selection matrices
                    PtTg = []
                    Pts = []
                    for c in range(2):
                        Pt = moep.tile([128, 128], F32, tag=f"Pt{c}")
                        nc.vector.tensor_scalar(out=Pt, in0=iota_f,
                                                scalar1=slot_tok[:, c, e:e + 1],
                                                scalar2=None, op0=ALU.is_equal)
                        nc.vector.tensor_scalar_mul(Pt, Pt, km_t[:, c, e:e + 1])
                        if debug and lidx == 0 and e == 0 and c == 0:
                            nc.sync.dma_start(out=dbg_pt[:], in_=Pt)
                        Pts.append(Pt)
                        Ptg = moep.tile([128, 128], F32, tag="Ptg")
                        nc.vector.tensor_scalar(out=Ptg, in0=Pt,
                                                scalar1=gates[:, c, e:e + 1],
                                                scalar2=None, op0=ALU.mult)
                        psq = psC.tile([128, 128], F32, tag="c")
                        nc.tensor.transpose(psq, Ptg, ident)
                        PtT = moep.tile([128, 128], F32, tag=f"PtT{c}")
                        nc.vector.tensor_copy(out=PtT, in_=psq)
                        PtTg.append(PtT)
                    # gather: y2selT[d, s] = sum_t y2[t, d] Pt[t, s], one bank per kc
                    ysel = moep.tile([128, 4, 128], F32, tag="ysel")
                    for kc in range(4):
                        ps_ys = psC.tile([128, 128], F32, tag="c")
                        for c in range(2):
                            nc.tensor.matmul(ps_ys, y2[:, c, kc * 128:(kc + 1) * 128],
                                             Pts[c], start=(c == 0), stop=(c == 1))
                        nc.vector.tensor_copy(out=ysel[:, kc, :], in_=ps_ys)
                    if debug and lidx == 0 and e == 0:
                        nc.sync.dma_start(out=dbg_ysel[:], in_=ysel)
                    # expert FFN on the <=128 gathered tokens
                    b1_sb = moep.tile([128, 16], F32, tag="b1")
                    nc.sync.dma_start(out=b1_sb, in_=eb1[lidx, e].rearrange("(fc p) -> p fc", p=128))
                    hT = big2.tile([128, 16, 128], F32, tag="b2")
                    for fq in range(4):
                        w1q = stream.tile([128, 4, 512], F32, tag="w1")
                        nc.sync.dma_start(
                            out=w1q,
                            in_=ew1[lidx, e].rearrange("(kc p) f -> p kc f", p=128)
                            [:, :, fq * 512:(fq + 1) * 512])
                        for fm in range(4):
                            fc = fq * 4 + fm
                            ps = psC.tile([128, 128], F32, tag="c")
                            for kc in range(4):
                                nc.tensor.matmul(ps, w1q[:, kc, fm * 128:(fm + 1) * 128],
                                                 ysel[:, kc, :], start=(kc == 0), stop=(kc == 3))
                            nc.scalar.activation(out=hT[:, fc, :], in_=ps, func=AF.Relu,
                                                 bias=b1_sb[:, fc:fc + 1], scale=1.0)
                    ps_eo = psA.tile([128, 512], F32, tag="a")
                    for qq in range(4):
                        w2q = stream.tile([128, 4, 512], F32, tag="w1")
                        nc.sync.dma_start(
                            out=w2q,
                            in_=ew2[lidx, e].rearrange("(q c p) d -> q p c d", q=4, p=128)[qq])
                        for cc in range(4):
                            fc = qq * 4 + cc
                            nc.tensor.matmul(ps_eo, hT[:, fc, :], w2q[:, cc, :],
                                             start=(qq == 0 and cc == 0),
                                             stop=(qq == 3 and cc == 3))
                    eo_sel = smax.tile([128, 512], F32, tag="res")
                    nc.vector.tensor_copy(out=eo_sel, in_=ps_eo)
                    if debug and lidx == 0 and e == 0:
                        nc.sync.dma_start(out=dbg_eosel[:], in_=eo_sel)
                    # scatter + gate: eo_acc[c] += PtTg[c]^T-weighted rows of eo_sel
                    for c in range(2):
                        nc.tensor.matmul(eo_acc[c], PtTg[c], eo_sel,
                                         start=False, stop=(e == E - 1))
                moe_acc = eo_acc
                # --- residual + LN2 ---
                g_bc = bcast_vec(ln_g[lidx, 2], D)
                b_bc = bcast_vec(ln_b[lidx, 2], D)
                y3 = ypool.tile([128, 2, D], F32, tag="y")
                for c in range(2):
                    t = smax.tile([128, D], F32, tag="res")
                    nc.vector.tensor_add(out=t, in0=moe_acc[c], in1=y2[:, c, :])
                    if debug and lidx == 0:
                        nc.sync.dma_start(out=dbg_moe[c], in_=t)
                    _ln(nc, pools, y3[:, c, :], t, g_bc, b_bc, small)
                if debug:
                    nc.sync.dma_start(out=dbg[lidx, 2].rearrange("(c p) d -> p c d", p=128),
                                      in_=y3)
                y_chunk = y3

            nc.sync.dma_start(out=out_y.rearrange("(c p) d -> p c d", p=128), in_=y_chunk)

    nc.compile()
    return nc


def host_prep(inputs):
    """Build per-core in_maps from full inputs."""
    f32 = lambda x: np.ascontiguousarray(np.asarray(x), dtype=np.float32)
    ids = np.asarray(inputs["decoder_input_ids"])
    emb = f32(inputs["embed_table"])
    pos = f32(inputs["pos_table"])
    y0 = emb[ids] + pos[:T][None]                   # [B, T, D] fp32
    enc = f32(inputs["enc_out"])
    am = np.asarray(inputs["attention_mask"]) != 0  # [B, T]

    sa_w = f32(inputs["sa_w"]); sa_b = f32(inputs["sa_b"])
    ca_w = f32(inputs["ca_w"]); ca_b = f32(inputs["ca_b"])

    def pack(w, b):
        return dict(
            wkv=np.ascontiguousarray(np.concatenate([w[:, 1], w[:, 2]], axis=2)),
            bkv=np.ascontiguousarray(np.concatenate([b[:, 1], b[:, 2]], axis=1)),
            wq=np.ascontiguousarray(w[:, 0]), bq=np.ascontiguousarray(b[:, 0] * 0.125),
            wo=np.ascontiguousarray(w[:, 3]), bo=np.ascontiguousarray(b[:, 3]))

    sa = pack(sa_w, sa_b)
    ca = pack(ca_w, ca_b)
    shared = {
        "wkv_sa": sa["wkv"], "bkv_sa": sa["bkv"], "wq_sa": sa["wq"], "bq_sa": sa["bq"],
        "wo_sa": sa["wo"], "bo_sa": sa["bo"],
        "wkv_ca": ca["wkv"], "bkv_ca": ca["bkv"], "wq_ca": ca["wq"], "bq_ca": ca["bq"],
        "wo_ca": ca["wo"], "bo_ca": ca["bo"],
        "ln_g": f32(inputs["ln_g"]), "ln_b": f32(inputs["ln_b"]),
        "rw": f32(inputs["router_w"]), "rb": f32(inputs["router_b"]),
        "iota_f": np.tile(np.arange(128, dtype=np.float32), (128, 1)),
        "umat": np.triu(np.ones((NCHUNK, NCHUNK), np.float32), 1),
        "ew1": f32(inputs["exp_w1"]), "eb1": f32(inputs["exp_b1"]),
        "ew2": f32(inputs["exp_w2"]), "eb2": f32(inputs["exp_b2"]),
    }

    in_maps = []
    causal_row = np.arange(TB)[None, :]
    for c in range(N_CORES):
        b, q = c // 4, c % 4
        rows = np.arange(q * NCHUNK, (q + 1) * NCHUNK)[:, None]
        ok = (causal_row <= rows) & am[b][None, :]
        mask = np.where(ok, np.float32(0.0), np.float32(-1e9)).astype(np.float32)
        m = dict(shared)
        m["y0_chunk"] = np.ascontiguousarray(y0[b, q * NCHUNK:(q + 1) * NCHUNK])
        m["y0T_b"] = np.ascontiguousarray(y0[b].T)
        m["mask"] = mask
        m["encT_b"] = np.ascontiguousarray(enc[b].T)
        in_maps.append(m)
    return in_maps


def kernel(**inputs):
    if "nc" not in _cache:
        _cache["nc"] = build()
    nc = _cache["nc"]
    in_maps = host_prep(inputs)
    res = run_bass_kernel_spmd(nc, in_maps, list(range(N_CORES)))
    _cache["last_results"] = res

    y = np.empty((B, T, D), np.float32)
    probs = np.empty((L, B * T, E), np.float32)
    for c in range(N_CORES):
        b, q = c // 4, c % 4
        y[b, q * NCHUNK:(q + 1) * NCHUNK] = res.results[c]["out_y"]
        probs[:, b * T + q * NCHUNK:b * T + (q + 1) * NCHUNK] = res.results[c]["probs_out"]

    # load-balancing aux loss from device-produced router probs
    lb_total = np.float32(0.0)
    for lidx in range(L):
        p = probs[lidx]                              # [B*T, E]
        idx = np.argsort(-p, axis=-1, kind="stable")[:, :K]
        kmask = np.zeros_like(p)
        np.put_along_axis(kmask, idx, 1.0, axis=-1)
        f = kmask.mean(0, dtype=np.float32)
        pm = p.mean(0, dtype=np.float32)
        lb_total = np.float32(lb_total + np.float32(E) * np.float32(np.sum(f * pm, dtype=np.float32)))
    return y, lb_total


# revision 18
# speedup vs baseline: 1.2845x; 1.0194x over previous
"""Trainium2 Bass kernel for nn_Decoder (moe_routing): 4-layer decoder with
self-attn + cross-attn + top-2-of-8 MoE per layer.

Distribution (8 NeuronCores): token-sharded. Core c owns 256 tokens
(cores 0-3 = batch 0, cores 4-7 = batch 1, chunk q = c % 4). Cross-attention
and the MoE are fully token-local. Self-attention needs full-batch K/V, so
each layer starts with one grouped AllGather of y over the 4 cores of each
batch; K/V projections are recomputed per core (causality handled by an
additive mask, so all cores run an identical program).

The MoE is computed densely (all 8 experts on the core's 256 tokens) in fp32
and combined with the renormalized top-2 gates, exactly mirroring the
reference math. Expert weights stream from HBM (64 MB/layer/core).

All matmuls are plain fp32: the router's top-2 margins go down to ~1e-5, so
tf32-like fast modes (fp32r/bf16) flip expert selection and are not usable.

Host side: embedding + positional lookup, input layout prep (transposes,
concatenated K/V weights, mask construction), final output assembly and the
load-balancing aux loss (computed from the device-produced router probs).
"""
import os
import numpy as np

import concourse.bass as bass
import concourse.mybir as mybir
import concourse.tile as tile
from concourse import bacc
from concourse.bass_utils import run_bass_kernel_spmd
from concourse.masks import make_identity

F32 = mybir.dt.float32
AF = mybir.ActivationFunctionType
ALU = mybir.AluOpType

B, T, S = 2, 1024, 1024
D, H, L = 512, 8, 4
E, K, FF = 8, 2, 2048
DH = D // H
EPS = 1e-5
NCHUNK = 256            # tokens per core
TB = 1024               # tokens per batch
N_CORES = 8
GROUPS = [[0, 1, 2, 3], [4, 5, 6, 7]]

DEBUG = bool(int(os.environ.get("KERNEL_DEBUG", "0")))
N_LAYERS = int(os.environ.get("KERNEL_LAYERS", str(L)))

_cache = {}


def _ln(nc, pools, out_ap, in_ap, g_bcast, b_bcast, stats_pool):
    """LayerNorm over free dim (512) of [128, 512] in_ap -> out_ap."""
    st = stats_pool.tile([128, 6], F32, tag="st")
    nc.vector.bn_stats(out=st, in_=in_ap)
    mv = stats_pool.tile([128, 2], F32, tag="mv")
    nc.vector.bn_aggr(out=mv, in_=st)
    sq = stats_pool.tile([128, 1], F32, tag="sq")
    nc.scalar.activation(out=sq, in_=mv[:, 1:2], func=AF.Sqrt,
                         bias=pools["eps"][:, 0:1], scale=1.0)
    rstd = stats_pool.tile([128, 1], F32, tag="rstd")
    nc.vector.reciprocal(out=rstd, in_=sq)
    nc.vector.tensor_scalar(out=in_ap, in0=in_ap, scalar1=mv[:, 0:1], scalar2=rstd,
                            op0=ALU.subtract, op1=ALU.mult)
    nc.vector.tensor_mul(out=in_ap, in0=in_ap, in1=g_bcast)
    nc.vector.tensor_add(out=out_ap, in0=in_ap, in1=b_bcast)


def build(n_layers=N_LAYERS, debug=DEBUG):
    nc = bacc.Bacc(None, target_bir_lowering=False, num_devices=N_CORES)

    # ---- I/O ----
    inp = {}
    def din(name, shape, dtype=F32):
        inp[name] = nc.dram_tensor(name, list(shape), dtype, kind="ExternalInput")
        return inp[name]

    y0_chunk = din("y0_chunk", [NCHUNK, D])
    y0T_b = din("y0T_b", [D, TB])
    mask_in = din("mask", [NCHUNK, TB])
    encT_in = din("encT_b", [D, S])
    wkv_sa = din("wkv_sa", [L, D, 2 * D]); bkv_sa = din("bkv_sa", [L, 2 * D])
    wq_sa = din("wq_sa", [L, D, D]); bq_sa = din("bq_sa", [L, D])
    wo_sa = din("wo_sa", [L, D, D]); bo_sa = din("bo_sa", [L, D])
    wkv_ca = din("wkv_ca", [L, D, 2 * D]); bkv_ca = din("bkv_ca", [L, 2 * D])
    wq_ca = din("wq_ca", [L, D, D]); bq_ca = din("bq_ca", [L, D])
    wo_ca = din("wo_ca", [L, D, D]); bo_ca = din("bo_ca", [L, D])
    ln_g = din("ln_g", [L, 3, D]); ln_b = din("ln_b", [L, 3, D])
    rw = din("rw", [L, D, E]); rb = din("rb", [L, E])
    iota_in = din("iota_f", [128, 128])
    umat_in = din("umat", [NCHUNK, NCHUNK])
    ew1 = din("ew1", [L, E, D, FF]); eb1 = din("eb1", [L, E, FF])
    ew2 = din("ew2", [L, E, FF, D]); eb2 = din("eb2", [L, E, D])

    out_y = nc.dram_tensor("out_y", [NCHUNK, D], F32, kind="ExternalOutput")
    probs_out = nc.dram_tensor("probs_out", [L, NCHUNK, E], F32, kind="ExternalOutput")
    if debug:
        dbg = nc.dram_tensor("dbg", [L, 3, NCHUNK, D], F32, kind="ExternalOutput")
        dbg_cs = nc.dram_tensor("dbg_cs", [8, NCHUNK], F32, kind="ExternalOutput")
        dbg_slot = nc.dram_tensor("dbg_slot", [128, 2, E], F32, kind="ExternalOutput")
        dbg_ysel = nc.dram_tensor("dbg_ysel", [128, 4, 128], F32, kind="ExternalOutput")
        dbg_eosel = nc.dram_tensor("dbg_eosel", [128, 512], F32, kind="ExternalOutput")
        dbg_moe = nc.dram_tensor("dbg_moe", [2, 128, 512], F32, kind="ExternalOutput")
        dbg_pt = nc.dram_tensor("dbg_pt", [128, 128], F32, kind="ExternalOutput")

    with tile.TileContext(nc) as tc:
        import contextlib
        ctx = contextlib.ExitStack()
        with ctx:
            const = ctx.enter_context(tc.tile_pool(name="const", bufs=1))
            ypool = ctx.enter_context(tc.tile_pool(name="ypool", bufs=2))
            ycTp = ctx.enter_context(tc.tile_pool(name="ycTp", bufs=2))
            ybTp = ctx.enter_context(tc.tile_pool(name="ybTp", bufs=1))
            kTp = ctx.enter_context(tc.tile_pool(name="kTp", bufs=1))
            vtokp = ctx.enter_context(tc.tile_pool(name="vtokp", bufs=1))
            wproj = ctx.enter_context(tc.tile_pool(name="wproj", bufs=2))
            smax = ctx.enter_context(tc.tile_pool(name="smax", bufs=3))
            qavp = ctx.enter_context(tc.tile_pool(name="qavp", bufs=2))
            big2 = ctx.enter_context(tc.tile_pool(name="big2", bufs=1))
            stream = ctx.enter_context(tc.tile_pool(name="stream", bufs=2))
            moep = ctx.enter_context(tc.tile_pool(name="moep", bufs=2))
            atp = ctx.enter_context(tc.tile_pool(name="atp", bufs=2))
            bcast = ctx.enter_context(tc.tile_pool(name="bcast", bufs=3))
            small = ctx.enter_context(tc.tile_pool(name="small", bufs=4))
            dramp = ctx.enter_context(tc.tile_pool(name="dramp", bufs=2, space="DRAM"))
            psA = ctx.enter_context(tc.tile_pool(name="psA", bufs=2, space="PSUM"))
            psB = ctx.enter_context(tc.tile_pool(name="psB", bufs=2, space="PSUM"))
            psC = ctx.enter_context(tc.tile_pool(name="psC", bufs=2, space="PSUM"))

            pools = {}

            ident = const.tile([128, 128], F32)
            make_identity(nc, ident)
            eps_t = const.tile([128, 1], F32)
            nc.vector.memset(eps_t, EPS)
            pools["eps"] = eps_t

            mask_t = const.tile([128, 2, TB], F32)
            nc.sync.dma_start(out=mask_t, in_=mask_in.rearrange("(c p) t -> p c t", p=128))
            encT = const.tile([128, 4, S], F32)
            nc.sync.dma_start(out=encT, in_=encT_in.rearrange("(kc p) t -> p kc t", p=128))

            iota_f = const.tile([128, 128], F32)
            nc.sync.dma_start(out=iota_f, in_=iota_in[:])
            umat_t = const.tile([128, 2, NCHUNK], F32)
            nc.sync.dma_start(out=umat_t, in_=umat_in.rearrange("(c p) t -> p c t", p=128))

            y_chunk = ypool.tile([128, 2, D], F32, tag="y")
            nc.sync.dma_start(out=y_chunk, in_=y0_chunk.rearrange("(c p) d -> p c d", p=128))

            def transpose_chunk(src_y):
                """[128, 2, 512] token-major -> ycT [128, 4, 256] feature-major."""
                ycT = ycTp.tile([128, 4, NCHUNK], F32, tag="ycT")
                for kc in range(4):
                    ps = psC.tile([128, 256], F32, tag="c")
                    for c in range(2):
                        nc.tensor.transpose(ps[:, c * 128:(c + 1) * 128],
                                            src_y[:, c, kc * 128:(kc + 1) * 128], ident)
                    nc.vector.tensor_copy(out=ycT[:, kc, :], in_=ps)
                return ycT

            def bcast_vec(vec_ap, width):
                t = bcast.tile([128, width], F32, tag="bc")
                src = bass.AP(tensor=vec_ap.tensor, offset=vec_ap.offset,
                              ap=[[0, 128]] + [list(x) for x in vec_ap.ap])
                nc.sync.dma_start(out=t, in_=src)
                return t

            def attention(lidx, ycT_cur, ybT, kvT_w, bkv_w, wq_w, bq_w, wo_w, bo_w,
                          kv_src, kv_len, use_mask, gj, y_res):
                """Generic attention. kv_src: [128, 4, kv_len] feature-major y/enc.
                Returns new y_chunk tile (post-residual-LN, ln index gj)."""
                # --- weights ---
                wkvh = []
                for h2 in range(2):  # halves of [D, 2D] weight
                    wt = wproj.tile([128, 4, D], F32, tag="wp")
                    nc.sync.dma_start(
                        out=wt, in_=kvT_w[lidx].rearrange("(kc p) m -> p kc m", p=128)
                        [:, :, h2 * D:(h2 + 1) * D])
                    wkvh.append(wt)
                wqt = wproj.tile([128, 4, D], F32, tag="wp")
                nc.sync.dma_start(out=wqt, in_=wq_w[lidx].rearrange("(kc p) m -> p kc m", p=128))
                wot = wproj.tile([128, 4, D], F32, tag="wp")
                nc.sync.dma_start(out=wot, in_=wo_w[lidx].rearrange("(kc p) m -> p kc m", p=128))
                bkv_sb = small.tile([128, 8], F32, tag="bkv")
                nc.sync.dma_start(out=bkv_sb, in_=bkv_w[lidx].rearrange("(mc p) -> p mc", p=128))
                bq_sb = small.tile([128, 4], F32, tag="bq")
                nc.sync.dma_start(out=bq_sb, in_=bq_w[lidx].rearrange("(mc p) -> p mc", p=128))

                nkc = kv_len // 512  # 512-wide rhs chunks
                # --- K/V projections (feature-major) ---
                kT = kTp.tile([128, 4, kv_len], F32, tag="kT")
                vT = big2.tile([128, 4, kv_len], F32, tag="b2")
                for mc in range(8):
                    dst = kT if mc < 4 else vT
                    wt = wkvh[mc // 4]
                    mloc = mc % 4
                    ps = psA.tile([128, kv_len], F32, tag="a")
                    for f in range(nkc):
                        for kc in range(4):
                            nc.tensor.matmul(
                                ps[:, f * 512:(f + 1) * 512],
                                wt[:, kc, mloc * 128:(mloc + 1) * 128],
                                kv_src[:, kc, f * 512:(f + 1) * 512],
                                start=(kc == 0), stop=(kc == 3))
                    nc.scalar.activation(out=dst[:, mloc, :], in_=ps, func=AF.Identity,
                                         bias=bkv_sb[:, mc:mc + 1], scale=1.0)
                # --- V -> token-major ---
                v_tok = vtokp.tile([128, 8, D], F32, tag="vtok")
                for tk in range(kv_len // 128):
                    ps = psB.tile([128, 512], F32, tag="b")
                    for mv in range(4):
                        nc.tensor.transpose(ps[:, mv * 128:(mv + 1) * 128],
                                            vT[:, mv, tk * 128:(tk + 1) * 128], ident)
                    nc.vector.tensor_copy(out=v_tok[:, tk, :], in_=ps)
                # --- Q (scaled by 1/8, feature-major) ---
                qT = qavp.tile([128, 4, NCHUNK], F32, tag="qav")
                for mc in range(4):
                    ps = psC.tile([128, 256], F32, tag="c")
                    for kc in range(4):
                        nc.tensor.matmul(ps, wqt[:, kc, mc * 128:(mc + 1) * 128],
                                         ycT_cur[:, kc, :], start=(kc == 0), stop=(kc == 3))
                    nc.scalar.activation(out=qT[:, mc, :], in_=ps, func=AF.Identity,
                                         bias=bq_sb[:, mc:mc + 1], scale=0.125)
                # --- per head/chunk: scores, softmax, transpose, A@V ---
                avT = qavp.tile([128, 4, NCHUNK], F32, tag="qav")
                for h in range(H):
                    mc, po = h // 2, (h % 2) * 64
                    attnT = atp.tile([128, kv_len // 128, NCHUNK], F32, tag="at")
                    for c in range(2):
                        ps = psA.tile([128, kv_len], F32, tag="a")
                        for f in range(nkc):
                            nc.tensor.matmul(
                                ps[:, f * 512:(f + 1) * 512],
                                qT[po:po + 64, mc, c * 128:(c + 1) * 128],
                                kT[po:po + 64, mc, f * 512:(f + 1) * 512],
                                start=True, stop=True)
                        if use_mask:
                            s_t = smax.tile([128, kv_len], F32, tag="sm")
                            nc.vector.tensor_add(out=s_t, in0=ps, in1=mask_t[:, c, :])
                            src = s_t
                        else:
                            src = ps
                        rmax = small.tile([128, 1], F32, tag="rmax")
                        nc.vector.reduce_max(rmax, src, axis=mybir.AxisListType.X)
                        nmax = small.tile([128, 1], F32, tag="nmax")
                        nc.scalar.mul(out=nmax, in_=rmax, mul=-1.0)
                        p_t = smax.tile([128, kv_len], F32, tag="sm")
                        rsum = small.tile([128, 1], F32, tag="rsum")
                        nc.scalar.activation(out=p_t, in_=src, func=AF.Exp,
                                             bias=nmax, scale=1.0, accum_out=rsum)
                        rinv = small.tile([128, 1], F32, tag="rinv")
                        nc.vector.reciprocal(out=rinv, in_=rsum)
                        nc.vector.tensor_scalar_mul(p_t, p_t, rinv)
                        # transpose attn -> [sk, tq] into both-chunk attnT
                        for g4 in range(kv_len // 512):
                            ps2 = psB.tile([128, 512], F32, tag="b")
                            for i in range(4):
                                sk = g4 * 4 + i
                                nc.tensor.transpose(ps2[:, i * 128:(i + 1) * 128],
                                                    p_t[:, sk * 128:(sk + 1) * 128], ident)
                            nc.vector.tensor_copy(
                                out=attnT[:, g4 * 4:(g4 + 1) * 4, c * 128:(c + 1) * 128],
                                in_=ps2.rearrange("p (a t) -> p a t", a=4))
                    # A @ V -> avT[dh, tq] (both chunks at once)
                    psv = psC.tile([64, NCHUNK], F32, tag="c")
                    nsk = kv_len // 128
                    for sk in range(nsk):
                        nc.tensor.matmul(psv, v_tok[:, sk, h * 64:(h + 1) * 64],
                                         attnT[:, sk, :],
                                         start=(sk == 0), stop=(sk == nsk - 1))
                    nc.vector.tensor_copy(out=avT[po:po + 64, mc, :], in_=psv)
                # --- out proj + bias + residual + LN ---
                bo_bc = bcast_vec(bo_w[lidx], D)
                g_bc = bcast_vec(ln_g[lidx, gj], D)
                b_bc = bcast_vec(ln_b[lidx, gj], D)
                y_new = ypool.tile([128, 2, D], F32, tag="y")
                for c in range(2):
                    ps = psB.tile([128, 512], F32, tag="b")
                    for kc in range(4):
                        nc.tensor.matmul(ps, avT[:, kc, c * 128:(c + 1) * 128],
                                         wot[:, kc, :], start=(kc == 0), stop=(kc == 3))
                    t = smax.tile([128, D], F32, tag="res")
                    nc.vector.tensor_add(out=t, in0=ps, in1=bo_bc)
                    nc.vector.tensor_add(out=t, in0=t, in1=y_res[:, c, :])
                    _ln(nc, pools, y_new[:, c, :], t, g_bc, b_bc, small)
                return y_new

            # ================= layers =================
            for lidx in range(n_layers):
                ycT = transpose_chunk(y_chunk)
                # --- gather y over the batch group ---
                if lidx == 0:
                    ybT = ybTp.tile([128, 4, TB], F32, tag="ybT")
                    nc.sync.dma_start(out=ybT,
                                      in_=y0T_b.rearrange("(kc p) t -> p kc t", p=128))
                else:
                    ag_in = dramp.tile([D, NCHUNK], F32, tag="agin")
                    nc.sync.dma_start(out=ag_in.rearrange("(kc p) t -> p kc t", p=128),
                                      in_=ycT)
                    ag_out = dramp.tile([4 * D, NCHUNK], F32, tag="agout")
                    nc.gpsimd.collective_compute(
                        "AllGather", ALU.bypass, replica_groups=GROUPS,
                        ins=[ag_in[:]], outs=[ag_out[:]])
                    ybT = ybTp.tile([128, 4, TB], F32, tag="ybT")
                    ag_v = ag_out.rearrange("(r c p) t -> r p c t", r=4, c=4, p=128)
                    for r in range(4):
                        nc.sync.dma_start(out=ybT[:, :, r * NCHUNK:(r + 1) * NCHUNK],
                                          in_=ag_v[r])

                # --- self-attention + LN0 ---
                y1 = attention(lidx, ycT, ybT, wkv_sa, bkv_sa, wq_sa, bq_sa,
                               wo_sa, bo_sa, ybT, TB, True, 0, y_chunk)
                if debug:
                    nc.sync.dma_start(out=dbg[lidx, 0].rearrange("(c p) d -> p c d", p=128),
                                      in_=y1)
                # --- cross-attention + LN1 ---
                y1cT = transpose_chunk(y1)
                y2 = attention(lidx, y1cT, None, wkv_ca, bkv_ca, wq_ca, bq_ca,
                               wo_ca, bo_ca, encT, S, False, 1, y1)
                if debug:
                    nc.sync.dma_start(out=dbg[lidx, 1].rearrange("(c p) d -> p c d", p=128),
                                      in_=y2)
                y2cT = transpose_chunk(y2)

                # --- router ---
                rw_t = small.tile([128, 4, E], F32, tag="rw")
                nc.sync.dma_start(out=rw_t, in_=rw[lidx].rearrange("(kc p) e -> p kc e", p=128))
                rb_ap = rb[lidx]
                rb_bc = small.tile([128, E], F32, tag="rbbc")
                nc.sync.dma_start(out=rb_bc, in_=bass.AP(
                    tensor=rb_ap.tensor, offset=rb_ap.offset,
                    ap=[[0, 128]] + [list(x) for x in rb_ap.ap]))
                probs = small.tile([128, 2, E], F32, tag="probs")
                gates = small.tile([128, 2, E], F32, tag="gates")
                m8_all = []
                for c in range(2):
                    ps = psC.tile([128, E], F32, tag="c")
                    for kc in range(4):
                        nc.tensor.matmul(ps, y2cT[:, kc, c * 128:(c + 1) * 128], rw_t[:, kc, :],
                                         start=(kc == 0), stop=(kc == 3))
                    logit = small.tile([128, E], F32, tag="logit")
                    nc.vector.tensor_add(out=logit, in0=ps, in1=rb_bc)
                    rmax = small.tile([128, 1], F32, tag="rmax")
                    nc.vector.reduce_max(rmax, logit, axis=mybir.AxisListType.X)
                    nmax = small.tile([128, 1], F32, tag="nmax")
                    nc.scalar.mul(out=nmax, in_=rmax, mul=-1.0)
                    rsum = small.tile([128, 1], F32, tag="rsum")
                    nc.scalar.activation(out=probs[:, c, :], in_=logit, func=AF.Exp,
                                         bias=nmax, scale=1.0, accum_out=rsum)
                    rinv = small.tile([128, 1], F32, tag="rinv")
                    nc.vector.reciprocal(out=rinv, in_=rsum)
                    nc.vector.tensor_scalar_mul(probs[:, c, :], probs[:, c, :], rinv)
                    # top-2 gates
                    m8 = small.tile([128, 8], F32, tag="m8", name=f"m8_{c}")
                    nc.vector.max(out=m8, in_=probs[:, c, :])
                    m8_all.append(m8[:, 1:2])
                    vs = small.tile([128, 1], F32, tag="vs")
                    nc.vector.tensor_add(out=vs, in0=m8[:, 0:1], in1=m8[:, 1:2])
                    gv = small.tile([128, 1], F32, tag="gv")
                    nc.vector.reciprocal(out=gv, in_=vs)
                    km = small.tile([128, E], F32, tag="km")
                    nc.vector.tensor_scalar(out=km, in0=probs[:, c, :], scalar1=m8[:, 1:2],
                                            scalar2=None, op0=ALU.is_ge)
                    nc.vector.tensor_mul(out=gates[:, c, :], in0=probs[:, c, :], in1=km)
                    nc.vector.tensor_scalar_mul(gates[:, c, :], gates[:, c, :], gv)
                nc.sync.dma_start(out=probs_out[lidx].rearrange("(c p) e -> p c e", p=128),
                                  in_=probs)

                # --- sparse top-2 MoE via permutation matmuls ---
                # exclusive prefix count of selected tokens per expert:
                # csT[e, t] = sum_{t'<t} kmask[t', e]   (umat[t',t] = 1 iff t'<t)
                km_t = moep.tile([128, 2, E], F32, tag="kmt")
                for c in range(2):
                    nc.vector.tensor_scalar(out=km_t[:, c, :], in0=probs[:, c, :],
                                            scalar1=m8_all[c], scalar2=None, op0=ALU.is_ge)
                ps_cs = psC.tile([8, NCHUNK], F32, tag="c")
                for c in range(2):
                    nc.tensor.matmul(ps_cs, km_t[:, c, :], umat_t[:, c, :],
                                     start=(c == 0), stop=(c == 1))
                csT = moep.tile([8, NCHUNK], F32, tag="csT")
                nc.vector.tensor_copy(out=csT, in_=ps_cs)
                if debug and lidx == 0:
                    nc.sync.dma_start(out=dbg_cs[:], in_=csT)
                slot_tok = moep.tile([128, 2, E], F32, tag="slot")
                for c in range(2):
                    pst = psC.tile([128, 128], F32, tag="c")
                    nc.tensor.transpose(pst[:, 0:8], csT[0:8, c * 128:(c + 1) * 128],
                                        ident[0:8, 0:8])
                    nc.vector.tensor_copy(out=slot_tok[:, c, :], in_=pst[:, 0:8])
                if debug and lidx == 0:
                    nc.sync.dma_start(out=dbg_slot[:], in_=slot_tok)
                # gates^T and b2 table for the Σ_e gate_e * b2_e term
                b2_all = moep.tile([8, D], F32, tag="b2a")
                nc.sync.dma_start(out=b2_all, in_=eb2[lidx])
                gT = moep.tile([8, NCHUNK], F32, tag="gT")
                for c in range(2):
                    psg = psC.tile([128, 128], F32, tag="c")
                    nc.tensor.transpose(psg[0:8, :], gates[:, c, :], ident)
                    nc.vector.tensor_copy(out=gT[:, c * 128:(c + 1) * 128], in_=psg[0:8, :])
                eo_acc = [psB.tile([128, 512], F32, tag="b", name=f"eoacc{_c}")
                          for _c in range(2)]
                for c in range(2):
                    nc.tensor.matmul(eo_acc[c], gT[0:8, c * 128:(c + 1) * 128], b2_all,
                                     start=True, stop=False)
                for e in range(E):
                    # selection matrices
                    PtTg = []
                    Pts = []
                    for c in range(2):
                        Pt = moep.tile([128, 128], F32, tag=f"Pt{c}")
                        nc.vector.tensor_scalar(out=Pt, in0=iota_f,
                                                scalar1=slot_tok[:, c, e:e + 1],
                                                scalar2=None, op0=ALU.is_equal)
                        nc.vector.tensor_scalar_mul(Pt, Pt, km_t[:, c, e:e + 1])
                        if debug and lidx == 0 and e == 0 and c == 0:
                            nc.sync.dma_start(out=dbg_pt[:], in_=Pt)
                        Pts.append(Pt)
                        Ptg = moep.tile([128, 128], F32, tag="Ptg")
                        nc.vector.tensor_scalar(out=Ptg, in0=Pt,
                                                scalar1=gates[:, c, e:e + 1],
                                                scalar2=None, op0=ALU.mult)
                        psq = psC.tile([128, 128], F32, tag="c")
                        nc.tensor.transpose(psq, Ptg, ident)
                        PtT = moep.tile([128, 128], F32, tag=f"PtT{c}")
                        nc.vector.tensor_copy(out=PtT, in_=psq)
                        PtTg.append(PtT)
                    # gather: y2selT[d, s] = sum_t y2[t, d] Pt[t, s], one bank per kc
                    ysel = moep.tile([128, 4, 128], F32, tag="ysel")
                    for kc in range(4):
                        ps_ys = psC.tile([128, 128], F32, tag="c")
                        for c in range(2):
                            nc.tensor.matmul(ps_ys, y2[:, c, kc * 128:(kc + 1) * 128],
                                             Pts[c], start=(c == 0), stop=(c == 1))
                        nc.vector.tensor_copy(out=ysel[:, kc, :], in_=ps_ys)
                    if debug and lidx == 0 and e == 0:
                        nc.sync.dma_start(out=dbg_ysel[:], in_=ysel)
                    # expert FFN on the <=128 gathered tokens.
                    # w1 runs token-major (16 wide matmuls instead of 64 narrow
                    # ones), then a PE transpose restores feature-major h with
                    # the bias+relu applied on the way back to SBUF.
                    b1_sb = moep.tile([128, 16], F32, tag="b1")
                    nc.sync.dma_start(out=b1_sb, in_=eb1[lidx, e].rearrange("(fc p) -> p fc", p=128))
                    hT = big2.tile([128, 16, 128], F32, tag="b2")
                    for fq in range(4):
                        w1q = stream.tile([128, 4, 512], F32, tag="w1")
                        nc.sync.dma_start(
                            out=w1q,
                            in_=ew1[lidx, e].rearrange("(kc p) f -> p kc f", p=128)
                            [:, :, fq * 512:(fq + 1) * 512])
                        psH = psA.tile([128, 512], F32, tag="a")
                        for kc in range(4):
                            nc.tensor.matmul(psH, ysel[:, kc, :], w1q[:, kc, :],
                                             start=(kc == 0), stop=(kc == 3))
                        h_tok = smax.tile([128, 512], F32, tag="res")
                        nc.vector.tensor_copy(out=h_tok, in_=psH)
                        for fm in range(4):
                            fc = fq * 4 + fm
                            pst = psC.tile([128, 128], F32, tag="c")
                            nc.tensor.transpose(pst, h_tok[:, fm * 128:(fm + 1) * 128], ident)
                            nc.scalar.activation(out=hT[:, fc, :], in_=pst, func=AF.Relu,
                                                 bias=b1_sb[:, fc:fc + 1], scale=1.0)
                    ps_eo = psA.tile([128, 512], F32, tag="a")
                    for qq in range(4):
                        w2q = stream.tile([128, 4, 512], F32, tag="w1")
                        nc.sync.dma_start(
                            out=w2q,
                            in_=ew2[lidx, e].rearrange("(q c p) d -> q p c d", q=4, p=128)[qq])
                        for cc in range(4):
                            fc = qq * 4 + cc
                            nc.tensor.matmul(ps_eo, hT[:, fc, :], w2q[:, cc, :],
                                             start=(qq == 0 and cc == 0),
                                             stop=(qq == 3 and cc == 3))
                    eo_sel = smax.tile([128, 512], F32, tag="res")
                    nc.vector.tensor_copy(out=eo_sel, in_=ps_eo)
                    if debug and lidx == 0 and e == 0:
                        nc.sync.dma_start(out=dbg_eosel[:], in_=eo_sel)
                    # scatter + gate: eo_acc[c] += PtTg[c]^T-weighted rows of eo_sel
                    for c in range(2):
                        nc.tensor.matmul(eo_acc[c], PtTg[c], eo_sel,
                                         start=False, stop=(e == E - 1))
                moe_acc = eo_acc
                # --- residual + LN2 ---
                g_bc = bcast_vec(ln_g[lidx, 2], D)
                b_bc = bcast_vec(ln_b[lidx, 2], D)
                y3 = ypool.tile([128, 2, D], F32, tag="y")
                for c in range(2):
                    t = smax.tile([128, D], F32, tag="res")
                    nc.vector.tensor_add(out=t, in0=moe_acc[c], in1=y2[:, c, :])
                    if debug and lidx == 0:
                        nc.sync.dma_start(out=dbg_moe[c], in_=t)
                    _ln(nc, pools, y3[:, c, :], t, g_bc, b_bc, small)
                if debug:
                    nc.sync.dma_start(out=dbg[lidx, 2].rearrange("(c p) d -> p c d", p=128),
                                      in_=y3)
                y_chunk = y3

            nc.sync.dma_start(out=out_y.rearrange("(c p) d -> p c d", p=128), in_=y_chunk)

    nc.compile()
    return nc


def host_prep(inputs):
    """Build per-core in_maps from full inputs."""
    f32 = lambda x: np.ascontiguousarray(np.asarray(x), dtype=np.float32)
    ids = np.asarray(inputs["decoder_input_ids"])
    emb = f32(inputs["embed_table"])
    pos = f32(inputs["pos_table"])
    y0 = emb[ids] + pos[:T][None]                   # [B, T, D] fp32
    enc = f32(inputs["enc_out"])
    am = np.asarray(inputs["attention_mask"]) != 0  # [B, T]

    sa_w = f32(inputs["sa_w"]); sa_b = f32(inputs["sa_b"])
    ca_w = f32(inputs["ca_w"]); ca_b = f32(inputs["ca_b"])

    def pack(w, b):
        return dict(
            wkv=np.ascontiguousarray(np.concatenate([w[:, 1], w[:, 2]], axis=2)),
            bkv=np.ascontiguousarray(np.concatenate([b[:, 1], b[:, 2]], axis=1)),
            wq=np.ascontiguousarray(w[:, 0]), bq=np.ascontiguousarray(b[:, 0] * 0.125),
            wo=np.ascontiguousarray(w[:, 3]), bo=np.ascontiguousarray(b[:, 3]))

    sa = pack(sa_w, sa_b)
    ca = pack(ca_w, ca_b)
    shared = {
        "wkv_sa": sa["wkv"], "bkv_sa": sa["bkv"], "wq_sa": sa["wq"], "bq_sa": sa["bq"],
        "wo_sa": sa["wo"], "bo_sa": sa["bo"],
        "wkv_ca": ca["wkv"], "bkv_ca": ca["bkv"], "wq_ca": ca["wq"], "bq_ca": ca["bq"],
        "wo_ca": ca["wo"], "bo_ca": ca["bo"],
        "ln_g": f32(inputs["ln_g"]), "ln_b": f32(inputs["ln_b"]),
        "rw": f32(inputs["router_w"]), "rb": f32(inputs["router_b"]),
        "iota_f": np.tile(np.arange(128, dtype=np.float32), (128, 1)),
        "umat": np.triu(np.ones((NCHUNK, NCHUNK), np.float32), 1),
        "ew1": f32(inputs["exp_w1"]), "eb1": f32(inputs["exp_b1"]),
        "ew2": f32(inputs["exp_w2"]), "eb2": f32(inputs["exp_b2"]),
    }

    in_maps = []
    causal_row = np.arange(TB)[None, :]
    for c in range(N_CORES):
        b, q = c // 4, c % 4
        rows = np.arange(q * NCHUNK, (q + 1) * NCHUNK)[:, None]
        ok = (causal_row <= rows) & am[b][None, :]
        mask = np.where(ok, np.float32(0.0), np.float32(-1e9)).astype(np.float32)
        m = dict(shared)
        m["y0_chunk"] = np.ascontiguousarray(y0[b, q * NCHUNK:(q + 1) * NCHUNK])
        m["y0T_b"] = np.ascontiguousarray(y0[b].T)
        m["mask"] = mask
        m["encT_b"] = np.ascontiguousarray(enc[b].T)
        in_maps.append(m)
    return in_maps


def kernel(**inputs):
    if "nc" not in _cache:
        _cache["nc"] = build()
    nc = _cache["nc"]
    in_maps = host_prep(inputs)
    res = run_bass_kernel_spmd(nc, in_maps, list(range(N_CORES)))
    _cache["last_results"] = res

    y = np.empty((B, T, D), np.float32)
    probs = np.empty((L, B * T, E), np.float32)
    for c in range(N_CORES):
        b, q = c // 4, c % 4
        y[b, q * NCHUNK:(q + 1) * NCHUNK] = res.results[c]["out_y"]
        probs[:, b * T + q * NCHUNK:b * T + (q + 1) * NCHUNK] = res.results[c]["probs_out"]

    # load-balancing aux loss from device-produced router probs
    lb_total = np.float32(0.0)
    for lidx in range(L):
        p = probs[lidx]                              # [B*T, E]
        idx = np.argsort(-p, axis=-1, kind="stable")[:, :K]
        kmask = np.zeros_like(p)
        np.put_along_axis(kmask, idx, 1.0, axis=-1)
        f = kmask.mean(0, dtype=np.float32)
        pm = p.mean(0, dtype=np.float32)
        lb_total = np.float32(lb_total + np.float32(E) * np.float32(np.sum(f * pm, dtype=np.float32)))
    return y, lb_total
